# revision 1
# baseline (speedup 1.0000x reference)
"""Trainium2 Bass kernel for CausalSelfAttention with KV cache (B=8, T=64, C=2048,
H=16, hd=128, S=4096), tensor-parallel over heads across 8 NeuronCores.

Sharding: 2 heads per core. Per core the device computes
  qkvT = W_attn_slice^T @ x^T           (transposed-activation layout, bf16)
  scoresT[s, t] = K^T-tiles (stationary) @ qT  -> exp on ScalarE -> attnT (bf16)
  l[t] = ones^T @ attnT  (PE partition-sum),  yT = V-tiles^T @ attnT / l
  yp = W_proj_rows^T @ yT               (partial over channels)
Host pre-transposes/pre-shards all inputs into exact SBUF layouts (bf16) and
reduces the 8 partial yp outputs + assembles the concatenated KV cache.
"""
import numpy as np
import ml_dtypes

bf16np = ml_dtypes.bfloat16
B, T, C, H, HD, S = 8, 64, 2048, 16, 128, 4096
NCORES, HC = 8, 2
NBH = B * HC          # 16 (b, hh) pairs per core
TOK = B * T           # 512
NT = S // 128         # 32 past s-tiles
NJ = NT + 1           # 33 s-tiles including new tokens

_prog_cache = {}


def _build_program():
    import concourse.mybir as mybir
    import concourse.tile as tile
    from concourse import bacc
    from concourse.bass import ts

    f32 = mybir.dt.float32
    bf16 = mybir.dt.bfloat16
    Exp = mybir.ActivationFunctionType.Exp
    mult = mybir.AluOpType.mult

    nc = bacc.Bacc("TRN2", target_bir_lowering=False, debug=False, num_devices=NCORES)

    kt_d = nc.dram_tensor("kt", [NBH, 128, S], bf16, kind="ExternalInput")
    v_d = nc.dram_tensor("v", [NBH, 128, NT, 128], bf16, kind="ExternalInput")
    xt_d = nc.dram_tensor("xt", [128, 16, TOK], bf16, kind="ExternalInput")
    wqkv_d = nc.dram_tensor("wqkv", [128, 16, 768], bf16, kind="ExternalInput")
    wproj_d = nc.dram_tensor("wproj", [128, HC, C], bf16, kind="ExternalInput")
    ident_d = nc.dram_tensor("ident", [128, 128], bf16, kind="ExternalInput")
    ones_d = nc.dram_tensor("ones", [128, 128], bf16, kind="ExternalInput")
    yp_d = nc.dram_tensor("yp", [128, 16, TOK], f32, kind="ExternalOutput")
    kvn_d = nc.dram_tensor("kvn", [128, 2, HC, TOK], bf16, kind="ExternalOutput")

    with tile.TileContext(nc) as tc:
        with (
            tc.tile_pool(name="const", bufs=1) as cpool,
            tc.tile_pool(name="work", bufs=2) as wpool,
        ):
            xt = cpool.tile([128, 16, TOK], bf16)
            nc.sync.dma_start(xt[:], xt_d[:])
            wqkv = cpool.tile([128, 16, 768], bf16)
            nc.sync.dma_start(wqkv[:], wqkv_d[:])
            wproj = cpool.tile([128, HC, C], bf16)
            nc.sync.dma_start(wproj[:], wproj_d[:])
            ident = cpool.tile([128, 128], bf16)
            nc.sync.dma_start(ident[:], ident_d[:])
            ones = cpool.tile([128, 128], bf16)
            nc.sync.dma_start(ones[:], ones_d[:])

            qt = cpool.tile([128, HC, TOK], bf16)     # [d, hh, tok]
            ktn = cpool.tile([128, HC, TOK], bf16)
            vtn = cpool.tile([128, HC, TOK], bf16)
            vn = cpool.tile([128, NBH, 128], bf16)    # v_new (t, d), rows 64+ zero
            yt = cpool.tile([128, HC, TOK], bf16)     # yT [d, hh, tok]

            nc.vector.memset(vn[:], 0.0)

            # ---------------- phase A: fused qkv projection (transposed) ----
            with tc.tile_pool(name="psA", bufs=2, space="PSUM") as psA:
                dsts = [(qt, 0), (qt, 1), (ktn, 0), (ktn, 1), (vtn, 0), (vtn, 1)]
                for j in range(6):
                    ps = psA.tile([128, TOK], f32, tag="qkv")
                    for ct in range(16):
                        nc.tensor.matmul(
                            ps[:], wqkv[:, ct, ts(j, 128)], xt[:, ct, :],
                            start=(ct == 0), stop=(ct == 15),
                        )
                    dst, hh = dsts[j]
                    nc.scalar.copy(dst[:, hh, :], ps[:])
                nc.sync.dma_start(kvn_d[:, 0], ktn[:])
                nc.sync.dma_start(kvn_d[:, 1], vtn[:])
                # transpose v_new tiles: (d, t) -> (t, d)
                for i in range(NBH):
                    b_, hh = divmod(i, HC)
                    pst = psA.tile([64, 128], bf16, tag="tp")
                    nc.tensor.transpose(pst[:], vtn[:, hh, ts(b_, 64)], ident[:])
                    nc.vector.tensor_copy(vn[:64, i, :], pst[:])

            # ---------------- phase B: attention per (b, hh) ----------------
            with (
                tc.tile_pool(name="psS", bufs=3, space="PSUM") as psS,
                tc.tile_pool(name="psY", bufs=2, space="PSUM") as psY,
                tc.tile_pool(name="psL", bufs=2, space="PSUM") as psL,
            ):
                for i in range(NBH):
                    b_, hh = divmod(i, HC)
                    ktt = wpool.tile([128, S], bf16, tag="kt")
                    nc.sync.dma_start(ktt[:], kt_d[i])
                    vt = wpool.tile([128, NT, 128], bf16, tag="v")
                    nc.sync.dma_start(vt[:], v_d[i])
                    at = wpool.tile([128, NJ * 64], bf16, tag="at")  # [s%128, (j, t)]
                    qv = qt[:, hh, ts(b_, 64)]

                    # scoresT chunks of 8 s-tiles -> exp -> attnT
                    for c in range(4):
                        ps = psS.tile([128, 512], f32, tag="sc")
                        for jj in range(8):
                            j = c * 8 + jj
                            nc.tensor.matmul(
                                ps[:, ts(jj, 64)], ktt[:, ts(j, 128)], qv,
                                start=True, stop=True,
                            )
                        nc.scalar.activation(
                            at[:, c * 512:(c + 1) * 512], ps[:], Exp
                        )
                    # new-token tile (33rd): k = 128 (d), m = 64 new tokens
                    ps = psS.tile([128, 512], f32, tag="sc")
                    nc.tensor.matmul(
                        ps[:64, :64], ktn[:, hh, ts(b_, 64)], qv,
                        start=True, stop=True,
                    )
                    nc.scalar.activation(at[:64, NT * 64:], ps[:64, :64], Exp)
                    nc.vector.memset(at[64:, NT * 64:], 0.0)

                    # l = sum_s attnT (PE ones trick), yT = V^T @ attnT
                    psl = psL.tile([128, 64], f32, tag="l")
                    for j in range(NJ):
                        nc.tensor.matmul(
                            psl[:], ones[:], at[:, ts(j, 64)],
                            start=(j == 0), stop=(j == NJ - 1),
                        )
                    psy = psY.tile([128, 64], f32, tag="yt")
                    for j in range(NJ):
                        lhsT = vt[:, j, :] if j < NT else vn[:, i, :]
                        nc.tensor.matmul(
                            psy[:], lhsT, at[:, ts(j, 64)],
                            start=(j == 0), stop=(j == NJ - 1),
                        )
                    rl = wpool.tile([128, 64], f32, tag="rl")
                    nc.vector.reciprocal(rl[:], psl[:])
                    nc.vector.tensor_tensor(
                        yt[:, hh, ts(b_, 64)], psy[:], rl[:], mult
                    )

                # ---------------- phase C: output projection ----------------
                for g in range(4):
                    st = wpool.tile([128, 4, TOK], f32, tag="ypst")
                    for k4 in range(4):
                        ot = g * 4 + k4
                        ps = psS.tile([128, 512], f32, tag="sc")
                        for cl in range(HC):
                            nc.tensor.matmul(
                                ps[:], wproj[:, cl, ts(ot, 128)], yt[:, cl, :],
                                start=(cl == 0), stop=(cl == HC - 1),
                            )
                        nc.scalar.copy(st[:, k4, :], ps[:])
                    nc.sync.dma_start(yp_d[:, ts(g, 4), :], st[:])

    nc.compile()
    return nc


def _host_prep(x, past_k, past_v, W_attn, W_proj):
    scale = 1.0 / np.sqrt(HD)
    x2 = np.ascontiguousarray(x.reshape(TOK, C))
    xt = np.ascontiguousarray(
        x2.T.reshape(16, 128, TOK).transpose(1, 0, 2)
    ).astype(bf16np)
    ident = np.eye(128, dtype=bf16np)
    ones = np.ones((128, 128), dtype=bf16np)

    in_maps = []
    for c in range(NCORES):
        h0, h1 = 2 * c, 2 * c + 1
        kt_c = np.ascontiguousarray(
            past_k[:, h0:h1 + 1].transpose(0, 1, 3, 2).reshape(NBH, 128, S)
        ).astype(bf16np)
        v_c = np.ascontiguousarray(
            past_v[:, h0:h1 + 1].reshape(B, HC, NT, 128, 128)
            .transpose(0, 1, 3, 2, 4).reshape(NBH, 128, NT, 128)
        ).astype(bf16np)
        cols = []
        for blk, sc in ((0, scale), (1, 1.0), (2, 1.0)):
            for h in (h0, h1):
                w = W_attn[:, blk * C + h * 128: blk * C + (h + 1) * 128]
                cols.append(w * sc if sc != 1.0 else w)
        wqkv = np.ascontiguousarray(
            np.concatenate(cols, axis=1).reshape(16, 128, 768).transpose(1, 0, 2)
        ).astype(bf16np)
        wproj = np.ascontiguousarray(
            W_proj[h0 * 128:(h1 + 1) * 128, :].reshape(HC, 128, C).transpose(1, 0, 2)
        ).astype(bf16np)
        in_maps.append({
            "kt": kt_c, "v": v_c, "xt": xt, "wqkv": wqkv, "wproj": wproj,
            "ident": ident, "ones": ones,
        })
    return in_maps


def _host_gather(results, past_k, past_v):
    ysum = np.zeros((128, 16, TOK), np.float32)
    for c in range(NCORES):
        ysum += results[c]["yp"]
    y = ysum.transpose(1, 0, 2).reshape(C, TOK).T.reshape(B, T, C)
    y = np.ascontiguousarray(y)

    k_new = np.empty((B, H, T, HD), np.float32)
    v_new = np.empty((B, H, T, HD), np.float32)
    for c in range(NCORES):
        kvn = results[c]["kvn"].astype(np.float32)   # (128, 2, HC, TOK)
        for hh in range(HC):
            k_new[:, 2 * c + hh] = kvn[:, 0, hh].T.reshape(B, T, HD)
            v_new[:, 2 * c + hh] = kvn[:, 1, hh].T.reshape(B, T, HD)
    k = np.concatenate([past_k, k_new], axis=2)
    v = np.concatenate([past_v, v_new], axis=2)
    return y, k, v


def kernel(x, past_k, past_v, W_attn, W_proj):
    from concourse.bass_utils import run_bass_kernel_spmd

    x = np.asarray(x, np.float32)
    past_k = np.asarray(past_k, np.float32)
    past_v = np.asarray(past_v, np.float32)
    W_attn = np.asarray(W_attn, np.float32)
    W_proj = np.asarray(W_proj, np.float32)

    if "nc" not in _prog_cache:
        _prog_cache["nc"] = _build_program()
    nc = _prog_cache["nc"]

    in_maps = _host_prep(x, past_k, past_v, W_attn, W_proj)
    res = run_bass_kernel_spmd(nc, in_maps, core_ids=list(range(NCORES)), trace=False)
    return _host_gather(res.results, past_k, past_v)


# revision 30
# speedup vs baseline: 71091.8100x; 71091.8100x over previous
"""Trainium2 Bass kernel for CausalSelfAttention with KV cache (B=8, T=64, C=2048,
H=16, hd=128, S=4096), tensor-parallel over heads across 8 NeuronCores.

Sharding: 2 heads per core. Per core the device computes
  qkvT = W_attn_slice^T @ x^T           (transposed-activation layout, bf16)
  scoresT[s, t] = K^T-tiles (stationary) @ qT  -> exp on ScalarE -> attnT (bf16)
  l[t] = ones^T @ attnT  (PE partition-sum),  yT = V-tiles^T @ attnT / l
  yp = W_proj_rows^T @ yT               (partial over channels)
Host pre-transposes/pre-shards all inputs into exact SBUF layouts (bf16) and
reduces the 8 partial yp outputs + assembles the concatenated KV cache.

DMA queueing: inputs stream on the SP HWDGE FIFO; outputs go on the ScalarE
HWDGE FIFO so a data-dependent output DMA never head-of-line-blocks input
prefetch.
"""
import numpy as np
import ml_dtypes

bf16np = ml_dtypes.bfloat16
B, T, C, H, HD, S = 8, 64, 2048, 16, 128, 4096
NCORES, HC = 8, 2
NBH = B * HC          # 16 (b, hh) pairs per core
TOK = B * T           # 512
NT = S // 128         # 32 past s-tiles
NJ = NT + 1           # 33 s-tiles including new tokens

_prog_cache = {}


def _build_program():
    import concourse.mybir as mybir
    import concourse.tile as tile
    from concourse import bacc
    from concourse.bass import ts

    f32 = mybir.dt.float32
    bf16 = mybir.dt.bfloat16
    Exp = mybir.ActivationFunctionType.Exp
    mult = mybir.AluOpType.mult
    addop = mybir.AluOpType.add
    axX = mybir.AxisListType.X

    nc = bacc.Bacc("TRN2", target_bir_lowering=False, debug=False, num_devices=NCORES)

    kt_d = nc.dram_tensor("kt", [NBH, 128, S], bf16, kind="ExternalInput")
    v_d = nc.dram_tensor("v", [NBH, 128, NT, 128], bf16, kind="ExternalInput")
    xt_d = nc.dram_tensor("xt", [128, 16, TOK], bf16, kind="ExternalInput")
    wqkv_d = nc.dram_tensor("wqkv", [128, 16, 768], bf16, kind="ExternalInput")
    wproj_d = nc.dram_tensor("wproj", [128, HC, C], bf16, kind="ExternalInput")
    ident_d = nc.dram_tensor("ident", [128, 128], bf16, kind="ExternalInput")
    ones_d = nc.dram_tensor("ones", [128, 128], bf16, kind="ExternalInput")
    yp_d = nc.dram_tensor("yp", [128, 2, 16, 256], bf16, kind="ExternalOutput")
    kvn_d = nc.dram_tensor("kvn", [128, 2, HC, TOK], bf16, kind="ExternalOutput")

    with tile.TileContext(nc) as tc:
        with (
            tc.tile_pool(name="const", bufs=1) as cpool,
            tc.tile_pool(name="work", bufs=2) as wpool,
            tc.tile_pool(name="ps", bufs=1, space="PSUM") as psum,
        ):
            xt = cpool.tile([128, 16, TOK], bf16)
            wqkv = cpool.tile([128, 16, 768], bf16)
            # interleave xt/wqkv chunk loads so qkv matmuls start early
            for c4 in range(4):
                sl = slice(c4 * 4, c4 * 4 + 4)
                nc.sync.dma_start(xt[:, sl, :], xt_d[:, sl, :])
                nc.sync.dma_start(wqkv[:, sl, :], wqkv_d[:, sl, :])
            wproj = cpool.tile([128, HC, C], bf16)
            nc.sync.dma_start(wproj[:], wproj_d[:])
            ident = cpool.tile([128, 128], bf16)
            nc.sync.dma_start(ident[:], ident_d[:])
            ones = cpool.tile([128, 128], bf16)
            nc.sync.dma_start(ones[:], ones_d[:])

            qt = cpool.tile([128, HC, TOK], bf16)     # [d, hh, tok]
            ktn = cpool.tile([128, HC, TOK], bf16)
            vtn = cpool.tile([128, HC, TOK], bf16)
            vn = cpool.tile([128, NBH, 128], bf16)    # v_new (t, d), rows 64+ zero
            yt = cpool.tile([128, HC, TOK], bf16)     # yT [d, hh, tok]

            nc.vector.memset(vn[:], 0.0)

            # ---------------- phase A: fused qkv projection (transposed) ----
            # two passes of 3 output col-tiles; ct-outer so matmuls chase the
            # chunked xt/wqkv DMAs instead of waiting for the full weights.
            # pass 0 completes everything head hh=0 needs.
            dsts = [[(qt, 0), (ktn, 0), (vtn, 0)], [(qt, 1), (ktn, 1), (vtn, 1)]]
            jmap = [[0, 2, 4], [1, 3, 5]]
            for p in range(2):
                pstiles = [psum.tile([128, TOK], f32, tag="sc", bufs=4,
                                     name=f"qkvps_{p}_{jj}")
                           for jj in range(3)]
                for ct in range(16):
                    for jj in range(3):
                        nc.tensor.matmul(
                            pstiles[jj][:], wqkv[:, ct, ts(jmap[p][jj], 128)],
                            xt[:, ct, :],
                            start=(ct == 0), stop=(ct == 15),
                        )
                for jj in range(3):
                    dst, hh = dsts[p][jj]
                    nc.scalar.copy(dst[:, hh, :], pstiles[jj][:])
                # transpose this head's v_new tiles: (d, t) -> (t, d)
                for b_ in range(B):
                    i = b_ * HC + p
                    pst = psum.tile([64, 128], bf16, tag="pj", bufs=2)
                    nc.tensor.transpose(pst[:], vtn[:, p, ts(b_, 64)], ident[:])
                    nc.vector.tensor_copy(vn[:64, i, :], pst[:])

            # ---------------- phases B/C: attention + projection ------------
            def emit_proj(half):
                tok_lo, tok_w = half * 256, 256
                st = wpool.tile([128, 16, tok_w], bf16, tag="ypst")
                for ot in range(16):
                    ps = psum.tile([128, 512], f32, tag="pj", bufs=2)
                    for cl in range(HC):
                        nc.tensor.matmul(
                            ps[:, :tok_w],
                            wproj[:, cl, ts(ot, 128)],
                            yt[:, cl, tok_lo:tok_lo + tok_w],
                            start=(cl == 0), stop=(cl == HC - 1),
                        )
                    eng = nc.vector.tensor_copy if ot % 2 else nc.scalar.copy
                    eng(st[:, ot, :], ps[:, :tok_w])
                    if ot == 7:
                        nc.gpsimd.dma_start(yp_d[:, half, :8], st[:, :8, :])
                nc.gpsimd.dma_start(yp_d[:, half, 8:], st[:, 8:, :])

            if True:
                for i in range(NBH):
                    b_, hh = divmod(i, HC)
                    ktt = wpool.tile([128, S], bf16, tag="kt", bufs=3)
                    nc.sync.dma_start(ktt[:], kt_d[i])
                    vt = wpool.tile([128, NT, 128], bf16, tag="v", bufs=3)
                    nc.sync.dma_start(vt[:], v_d[i])
                    at = wpool.tile([128, NJ * 64], bf16, tag="at")  # [s%128, (j, t)]
                    qv = qt[:, hh, ts(b_, 64)]

                    # scoresT chunks of 8 s-tiles -> exp -> attnT
                    for c in range(4):
                        ps = psum.tile([128, 512], f32, tag="sc", bufs=4)
                        for jj in range(8):
                            j = c * 8 + jj
                            nc.tensor.matmul(
                                ps[:, ts(jj, 64)], ktt[:, ts(j, 128)], qv,
                                start=True, stop=True,
                            )
                        nc.scalar.activation(
                            at[:, c * 512:(c + 1) * 512], ps[:], Exp
                        )
                    # new-token tile (33rd): k = 128 (d), m = 64 new tokens
                    ps = psum.tile([128, 512], f32, tag="sc", bufs=4)
                    nc.tensor.matmul(
                        ps[:64, :64], ktn[:, hh, ts(b_, 64)], qv,
                        start=True, stop=True,
                    )
                    nc.scalar.activation(at[:64, NT * 64:], ps[:64, :64], Exp)
                    nc.vector.memset(at[64:, NT * 64:], 0.0)

                    # l = sum_s attnT: chunked ones-matmuls accumulate partial
                    # sums grouped by (j mod 8); DVE reduces the 8 groups.
                    psl = psum.tile([128, 512], f32, tag="l", bufs=1)
                    for c in range(4):
                        nc.tensor.matmul(
                            psl[:], ones[:], at[:, c * 512:(c + 1) * 512],
                            start=(c == 0), stop=False,
                        )
                    nc.tensor.matmul(
                        psl[:, :64], ones[:], at[:, NT * 64:],
                        start=False, stop=True,
                    )
                    psy = psum.tile([128, 64], f32, tag="yt", bufs=1)
                    for j in range(NJ):
                        lhsT = vt[:, j, :] if j < NT else vn[:, i, :]
                        nc.tensor.matmul(
                            psy[:], lhsT, at[:, ts(j, 64)],
                            start=(j == 0), stop=(j == NJ - 1),
                        )
                    lsum = wpool.tile([128, 64], f32, tag="lsum")
                    nc.vector.tensor_reduce(
                        lsum[:], psl[:].rearrange("p (j t) -> p t j", j=8),
                        axX, addop,
                    )
                    rl = wpool.tile([128, 64], f32, tag="rl")
                    nc.vector.reciprocal(rl[:], lsum[:])
                    nc.vector.tensor_tensor(
                        yt[:, hh, ts(b_, 64)], psy[:], rl[:], mult
                    )

                    if i == 7:
                        emit_proj(0)                # tokens of b 0..3
                    if i == 1:
                        nc.gpsimd.dma_start(kvn_d[:, 0], ktn[:])
                        nc.gpsimd.dma_start(kvn_d[:, 1], vtn[:])

                emit_proj(1)                        # tokens of b 4..7

    nc.compile()
    return nc


def _host_prep(x, past_k, past_v, W_attn, W_proj):
    scale = np.float32(1.0 / np.sqrt(HD))
    x2 = np.ascontiguousarray(x.reshape(TOK, C))
    xt = np.ascontiguousarray(
        x2.T.reshape(16, 128, TOK).transpose(1, 0, 2)
    ).astype(bf16np)
    ident = np.eye(128, dtype=bf16np)
    ones = np.ones((128, 128), dtype=bf16np)

    # cast first (halves the bytes moved by the big transposes)
    pk8 = past_k.astype(bf16np)
    pv8 = past_v.astype(bf16np)

    in_maps = []
    for c in range(NCORES):
        h0, h1 = 2 * c, 2 * c + 1
        kt_c = np.ascontiguousarray(
            pk8[:, h0:h1 + 1].transpose(0, 1, 3, 2).reshape(NBH, 128, S)
        )
        v_c = np.ascontiguousarray(
            pv8[:, h0:h1 + 1].reshape(B, HC, NT, 128, 128)
            .transpose(0, 1, 3, 2, 4).reshape(NBH, 128, NT, 128)
        )
        cols = []
        for blk, sc in ((0, scale), (1, None), (2, None)):
            for h in (h0, h1):
                w = W_attn[:, blk * C + h * 128: blk * C + (h + 1) * 128]
                cols.append(w * sc if sc is not None else w)
        wqkv = np.ascontiguousarray(
            np.concatenate(cols, axis=1).reshape(16, 128, 768).transpose(1, 0, 2)
        ).astype(bf16np)
        wproj = np.ascontiguousarray(
            W_proj[h0 * 128:(h1 + 1) * 128, :].reshape(HC, 128, C).transpose(1, 0, 2)
        ).astype(bf16np)
        in_maps.append({
            "kt": kt_c, "v": v_c, "xt": xt, "wqkv": wqkv, "wproj": wproj,
            "ident": ident, "ones": ones,
        })
    return in_maps


def _host_gather(results, past_k, past_v):
    ysum = np.zeros((128, 2, 16, 256), np.float32)
    for c in range(NCORES):
        ysum += results[c]["yp"].astype(np.float32)
    # [p, half, ot, tq] -> yT[ot*128+p, half*256+tq]
    yT = ysum.transpose(2, 0, 1, 3).reshape(C, TOK)
    y = yT.T.reshape(B, T, C)
    y = np.ascontiguousarray(y)

    k_new = np.empty((B, H, T, HD), np.float32)
    v_new = np.empty((B, H, T, HD), np.float32)
    for c in range(NCORES):
        kvn = results[c]["kvn"].astype(np.float32)   # (128, 2, HC, TOK)
        for hh in range(HC):
            k_new[:, 2 * c + hh] = kvn[:, 0, hh].T.reshape(B, T, HD)
            v_new[:, 2 * c + hh] = kvn[:, 1, hh].T.reshape(B, T, HD)
    k = np.concatenate([past_k, k_new], axis=2)
    v = np.concatenate([past_v, v_new], axis=2)
    return y, k, v


def kernel(x, past_k, past_v, W_attn, W_proj):
    from concourse.bass_utils import run_bass_kernel_spmd

    x = np.asarray(x, np.float32)
    past_k = np.asarray(past_k, np.float32)
    past_v = np.asarray(past_v, np.float32)
    W_attn = np.asarray(W_attn, np.float32)
    W_proj = np.asarray(W_proj, np.float32)

    if "nc" not in _prog_cache:
        _prog_cache["nc"] = _build_program()
    nc = _prog_cache["nc"]

    in_maps = _host_prep(x, past_k, past_v, W_attn, W_proj)
    res = run_bass_kernel_spmd(nc, in_maps, core_ids=list(range(NCORES)), trace=False)
    return _host_gather(res.results, past_k, past_v)


# revision 39
# speedup vs baseline: 76062.4708x; 1.0699x over previous
"""Trainium2 Bass kernel for CausalSelfAttention with KV cache (B=8, T=64, C=2048,
H=16, hd=128, S=4096), tensor-parallel over heads across 8 NeuronCores.

Sharding: 2 heads per core. Per core the device computes
  qkvT = W_attn_slice^T @ x^T           (transposed-activation layout, bf16)
  scoresT[s, t] = K^T-tiles (stationary) @ qT  -> exp on ScalarE -> attnT (bf16)
  l[t] = ones^T @ attnT  (PE partition-sum),  yT = V-tiles^T @ attnT / l
  yp = W_proj_rows^T @ yT               (partial over channels)
Host pre-transposes/pre-shards all inputs into exact SBUF layouts (bf16) and
reduces the 8 partial yp outputs + assembles the concatenated KV cache.

DMA queueing: inputs stream on the SP HWDGE FIFO; outputs go on the ScalarE
HWDGE FIFO so a data-dependent output DMA never head-of-line-blocks input
prefetch.
"""
import numpy as np
import ml_dtypes

bf16np = ml_dtypes.bfloat16
B, T, C, H, HD, S = 8, 64, 2048, 16, 128, 4096
NCORES, HC = 8, 2
NBH = B * HC          # 16 (b, hh) pairs per core
TOK = B * T           # 512
NT = S // 128         # 32 past s-tiles
NJ = NT + 1           # 33 s-tiles including new tokens

_prog_cache = {}


def _build_program():
    import concourse.mybir as mybir
    import concourse.tile as tile
    from concourse import bacc
    from concourse.bass import ts
    import bass_rust

    f32 = mybir.dt.float32
    bf16 = mybir.dt.bfloat16
    Exp = mybir.ActivationFunctionType.Exp
    mult = mybir.AluOpType.mult
    addop = mybir.AluOpType.add
    axX = mybir.AxisListType.X

    nc = bacc.Bacc("TRN2", target_bir_lowering=False, debug=False, num_devices=NCORES)

    kt_d = nc.dram_tensor("kt", [NBH, 128, S], bf16, kind="ExternalInput")
    v_d = nc.dram_tensor("v", [NBH, 128, NT, 128], bf16, kind="ExternalInput")
    xt_d = nc.dram_tensor("xt", [128, 16, TOK], bf16, kind="ExternalInput")
    wqkv_d = nc.dram_tensor("wqkv", [128, 16, 768], bf16, kind="ExternalInput")
    wproj_d = nc.dram_tensor("wproj", [128, HC, C], bf16, kind="ExternalInput")
    ident_d = nc.dram_tensor("ident", [128, 128], bf16, kind="ExternalInput")
    ones_d = nc.dram_tensor("ones", [128, 128], bf16, kind="ExternalInput")
    yp_d = nc.dram_tensor("yp", [128, 2, 16, 256], bf16, kind="ExternalOutput")
    kvn_d = nc.dram_tensor("kvn", [128, 2, HC, TOK], bf16, kind="ExternalOutput")

    with tile.TileContext(nc) as tc:
        with (
            tc.tile_pool(name="const", bufs=1) as cpool,
            tc.tile_pool(name="work", bufs=2) as wpool,
            tc.tile_pool(name="ps", bufs=1, space="PSUM") as psum,
        ):
            xt = cpool.tile([128, 16, TOK], bf16)
            wqkv = cpool.tile([128, 16, 768], bf16)
            # interleave xt/wqkv chunk loads so qkv matmuls start early
            for c4 in range(4):
                sl = slice(c4 * 4, c4 * 4 + 4)
                nc.sync.dma_start(xt[:, sl, :], xt_d[:, sl, :])
                nc.sync.dma_start(wqkv[:, sl, :], wqkv_d[:, sl, :])
            wproj = cpool.tile([128, HC, C], bf16)
            nc.sync.dma_start(wproj[:], wproj_d[:])
            ident = cpool.tile([128, 128], bf16)
            nc.sync.dma_start(ident[:], ident_d[:])
            ones = cpool.tile([128, 128], bf16)
            nc.sync.dma_start(ones[:], ones_d[:])
            # last iteration's K, loaded up-front: its scores/exp/l-sum can
            # then run early, so only AV is gated on the final (V) transfer
            ktlast = cpool.tile([128, S], bf16)
            nc.sync.dma_start(ktlast[:], kt_d[NBH - 1])

            qt = cpool.tile([128, HC, TOK], bf16)     # [d, hh, tok]
            ktn = cpool.tile([128, HC, TOK], bf16)
            vtn = cpool.tile([128, HC, TOK], bf16)
            vn = cpool.tile([128, NBH, 128], bf16)    # v_new (t, d), rows 64+ zero
            yt = cpool.tile([128, HC, TOK], bf16)     # yT [d, hh, tok]

            nc.vector.memset(vn[:], 0.0)

            # ---------------- phase A: fused qkv projection (transposed) ----
            # two passes of 3 output col-tiles; ct-outer so matmuls chase the
            # chunked xt/wqkv DMAs instead of waiting for the full weights.
            # pass 0 completes everything head hh=0 needs.
            dsts = [[(qt, 0), (ktn, 0), (vtn, 0)], [(qt, 1), (ktn, 1), (vtn, 1)]]
            jmap = [[0, 2, 4], [1, 3, 5]]
            for p in range(2):
                pstiles = [psum.tile([128, TOK], f32, tag="sc", bufs=4,
                                     name=f"qkvps_{p}_{jj}")
                           for jj in range(3)]
                for ct in range(16):
                    for jj in range(3):
                        nc.tensor.matmul(
                            pstiles[jj][:], wqkv[:, ct, ts(jmap[p][jj], 128)],
                            xt[:, ct, :],
                            start=(ct == 0), stop=(ct == 15),
                        )
                for jj in range(3):
                    dst, hh = dsts[p][jj]
                    nc.scalar.copy(dst[:, hh, :], pstiles[jj][:])
                # transpose this head's v_new tiles: (d, t) -> (t, d)
                for b_ in range(B):
                    i = b_ * HC + p
                    pst = psum.tile([64, 128], bf16, tag="pj", bufs=2)
                    nc.tensor.transpose(pst[:], vtn[:, p, ts(b_, 64)], ident[:])
                    nc.vector.tensor_copy(vn[:64, i, :], pst[:])

            # ---------------- phases B/C: attention + projection ------------
            def emit_proj(half, dma_eng=None, ptag="pj", pbufs=2,
                          defer_dma=False):
                dma_eng = dma_eng or nc.gpsimd
                tok_lo, tok_w = half * 256, 256
                st = wpool.tile([128, 16, tok_w], bf16, tag="ypst")
                for op in range(8):
                    ps = psum.tile([128, 512], f32, tag=ptag, bufs=pbufs)
                    for sub in range(2):
                        ot = op * 2 + sub
                        for cl in range(HC):
                            nc.tensor.matmul(
                                ps[:, ts(sub, tok_w)],
                                wproj[:, cl, ts(ot, 128)],
                                yt[:, cl, tok_lo:tok_lo + tok_w],
                                start=(cl == 0), stop=(cl == HC - 1),
                            )
                    eng = nc.vector.tensor_copy if op % 2 else nc.scalar.copy
                    eng(st[:, ts(op, 2), :], ps[:])
                    if defer_dma:
                        continue
                    if op == 3:
                        dma_eng.dma_start(yp_d[:, half, :8], st[:, :8, :])
                    if op == 5:
                        dma_eng.dma_start(yp_d[:, half, 8:12], st[:, 8:12, :])
                if not defer_dma:
                    dma_eng.dma_start(yp_d[:, half, 12:], st[:, 12:, :])
                return st

            if True:
                for i in range(NBH):
                    b_, hh = divmod(i, HC)
                    if i == NBH - 1:
                        ktt = ktlast
                    else:
                        ktt = wpool.tile([128, S], bf16, tag="kt", bufs=3)
                        nc.sync.dma_start(ktt[:], kt_d[i])
                    vt = wpool.tile([128, NT, 128], bf16, tag="v", bufs=3)
                    vdma = nc.sync.dma_start(vt[:], v_d[i])
                    if i == NBH - 1:
                        last_in_dma = vdma
                    at = wpool.tile([128, NJ * 64], bf16, tag="at")  # [s%128, (j, t)]
                    qv = qt[:, hh, ts(b_, 64)]

                    # new-token tile (33rd) first: it has no DMA dependency
                    ps = psum.tile([128, 512], f32, tag="sc", bufs=4)
                    nc.tensor.matmul(
                        ps[:64, :64], ktn[:, hh, ts(b_, 64)], qv,
                        start=True, stop=True,
                    )
                    nc.scalar.activation(at[:64, NT * 64:], ps[:64, :64], Exp)
                    nc.vector.memset(at[64:, NT * 64:], 0.0)
                    psl = psum.tile([128, 512], f32, tag="l", bufs=1)
                    nc.tensor.matmul(
                        psl[:, :64], ones[:], at[:, NT * 64:],
                        start=True, stop=False,
                    )
                    # scoresT chunks of 8 s-tiles -> exp -> attnT, each chunk
                    # followed by its l-sum accumulation (grouped by j mod 8)
                    for c in range(4):
                        ps = psum.tile([128, 512], f32, tag="sc", bufs=4)
                        for jj in range(8):
                            j = c * 8 + jj
                            nc.tensor.matmul(
                                ps[:, ts(jj, 64)], ktt[:, ts(j, 128)], qv,
                                start=True, stop=True,
                            )
                        nc.scalar.activation(
                            at[:, c * 512:(c + 1) * 512], ps[:], Exp
                        )
                        nc.tensor.matmul(
                            psl[:], ones[:], at[:, c * 512:(c + 1) * 512],
                            start=False, stop=(c == 3),
                        )
                    psy = psum.tile([128, 64], f32, tag="yt", bufs=1)
                    for j in range(NJ):
                        lhsT = vt[:, j, :] if j < NT else vn[:, i, :]
                        nc.tensor.matmul(
                            psy[:], lhsT, at[:, ts(j, 64)],
                            start=(j == 0), stop=(j == NJ - 1),
                        )
                    lsum = wpool.tile([128, 64], f32, tag="lsum")
                    nc.vector.tensor_reduce(
                        lsum[:], psl[:].rearrange("p (j t) -> p t j", j=8),
                        axX, addop,
                    )
                    rl = wpool.tile([128, 64], f32, tag="rl")
                    nc.vector.reciprocal(rl[:], lsum[:])
                    nc.vector.tensor_tensor(
                        yt[:, hh, ts(b_, 64)], psy[:], rl[:], mult
                    )

                    if i == 7:
                        st_a = emit_proj(0, defer_dma=True)  # tokens of b 0..3

                # deferred output transfers, explicitly ordered after the
                # last input DMA so they fill the tail's DMA-idle window
                # instead of delaying input arrival
                for dst, src_ap in ((yp_d[:, 0], st_a[:]),
                                    (kvn_d[:, 0], ktn[:]),
                                    (kvn_d[:, 1], vtn[:])):
                    h = nc.gpsimd.dma_start(dst, src_ap)
                    bass_rust.add_dep_helper(
                        h.ins, last_in_dma.ins,
                        reason="defer output transfer past final input",
                    )
                emit_proj(1, dma_eng=nc.sync, ptag="sc", pbufs=4)  # b 4..7

    nc.compile()
    return nc


def _host_prep(x, past_k, past_v, W_attn, W_proj):
    scale = np.float32(1.0 / np.sqrt(HD))
    x2 = np.ascontiguousarray(x.reshape(TOK, C))
    xt = np.ascontiguousarray(
        x2.T.reshape(16, 128, TOK).transpose(1, 0, 2)
    ).astype(bf16np)
    ident = np.eye(128, dtype=bf16np)
    ones = np.ones((128, 128), dtype=bf16np)

    # cast first (halves the bytes moved by the big transposes)
    pk8 = past_k.astype(bf16np)
    pv8 = past_v.astype(bf16np)

    in_maps = []
    for c in range(NCORES):
        h0, h1 = 2 * c, 2 * c + 1
        kt_c = np.ascontiguousarray(
            pk8[:, h0:h1 + 1].transpose(0, 1, 3, 2).reshape(NBH, 128, S)
        )
        v_c = np.ascontiguousarray(
            pv8[:, h0:h1 + 1].reshape(B, HC, NT, 128, 128)
            .transpose(0, 1, 3, 2, 4).reshape(NBH, 128, NT, 128)
        )
        cols = []
        for blk, sc in ((0, scale), (1, None), (2, None)):
            for h in (h0, h1):
                w = W_attn[:, blk * C + h * 128: blk * C + (h + 1) * 128]
                cols.append(w * sc if sc is not None else w)
        wqkv = np.ascontiguousarray(
            np.concatenate(cols, axis=1).reshape(16, 128, 768).transpose(1, 0, 2)
        ).astype(bf16np)
        wproj = np.ascontiguousarray(
            W_proj[h0 * 128:(h1 + 1) * 128, :].reshape(HC, 128, C).transpose(1, 0, 2)
        ).astype(bf16np)
        in_maps.append({
            "kt": kt_c, "v": v_c, "xt": xt, "wqkv": wqkv, "wproj": wproj,
            "ident": ident, "ones": ones,
        })
    return in_maps


def _host_gather(results, past_k, past_v):
    ysum = np.zeros((128, 2, 16, 256), np.float32)
    for c in range(NCORES):
        ysum += results[c]["yp"].astype(np.float32)
    # [p, half, ot, tq] -> yT[ot*128+p, half*256+tq]
    yT = ysum.transpose(2, 0, 1, 3).reshape(C, TOK)
    y = yT.T.reshape(B, T, C)
    y = np.ascontiguousarray(y)

    k_new = np.empty((B, H, T, HD), np.float32)
    v_new = np.empty((B, H, T, HD), np.float32)
    for c in range(NCORES):
        kvn = results[c]["kvn"].astype(np.float32)   # (128, 2, HC, TOK)
        for hh in range(HC):
            k_new[:, 2 * c + hh] = kvn[:, 0, hh].T.reshape(B, T, HD)
            v_new[:, 2 * c + hh] = kvn[:, 1, hh].T.reshape(B, T, HD)
    k = np.concatenate([past_k, k_new], axis=2)
    v = np.concatenate([past_v, v_new], axis=2)
    return y, k, v


def kernel(x, past_k, past_v, W_attn, W_proj):
    from concourse.bass_utils import run_bass_kernel_spmd

    x = np.asarray(x, np.float32)
    past_k = np.asarray(past_k, np.float32)
    past_v = np.asarray(past_v, np.float32)
    W_attn = np.asarray(W_attn, np.float32)
    W_proj = np.asarray(W_proj, np.float32)

    if "nc" not in _prog_cache:
        _prog_cache["nc"] = _build_program()
    nc = _prog_cache["nc"]

    in_maps = _host_prep(x, past_k, past_v, W_attn, W_proj)
    res = run_bass_kernel_spmd(nc, in_maps, core_ids=list(range(NCORES)), trace=False)
    return _host_gather(res.results, past_k, past_v)


# revision 42
# speedup vs baseline: 76382.0327x; 1.0042x over previous
"""Trainium2 Bass kernel for CausalSelfAttention with KV cache (B=8, T=64, C=2048,
H=16, hd=128, S=4096), tensor-parallel over heads across 8 NeuronCores.

Sharding: 2 heads per core. Per core the device computes
  qkvT = W_attn_slice^T @ x^T           (transposed-activation layout, bf16)
  scoresT[s, t] = K^T-tiles (stationary) @ qT  -> exp on ScalarE -> attnT (bf16)
  l[t] = ones^T @ attnT  (PE partition-sum),  yT = V-tiles^T @ attnT / l
  yp = W_proj_rows^T @ yT               (partial over channels)
Host pre-transposes/pre-shards all inputs into exact SBUF layouts (bf16) and
reduces the 8 partial yp outputs + assembles the concatenated KV cache.

DMA queueing: inputs stream on the SP HWDGE FIFO; bulk outputs are emitted
after every input DMA on the same FIFO (order alone defers them past the
last input, filling the tail's DMA-idle window without delaying input
arrival). The last iteration's K is pinned up-front and its V transfer is
split in half so only a minimal AV+scale+projection chain trails the final
input bytes.
"""
import numpy as np
import ml_dtypes

bf16np = ml_dtypes.bfloat16
B, T, C, H, HD, S = 8, 64, 2048, 16, 128, 4096
NCORES, HC = 8, 2
NBH = B * HC          # 16 (b, hh) pairs per core
TOK = B * T           # 512
NT = S // 128         # 32 past s-tiles
NJ = NT + 1           # 33 s-tiles including new tokens

_prog_cache = {}


def _build_program():
    import concourse.mybir as mybir
    import concourse.tile as tile
    from concourse import bacc
    from concourse.bass import ts
    import bass_rust

    f32 = mybir.dt.float32
    bf16 = mybir.dt.bfloat16
    Exp = mybir.ActivationFunctionType.Exp
    mult = mybir.AluOpType.mult
    addop = mybir.AluOpType.add
    axX = mybir.AxisListType.X

    nc = bacc.Bacc("TRN2", target_bir_lowering=False, debug=False, num_devices=NCORES)

    kt_d = nc.dram_tensor("kt", [NBH, 128, S], bf16, kind="ExternalInput")
    v_d = nc.dram_tensor("v", [NBH, 128, NT, 128], bf16, kind="ExternalInput")
    xt_d = nc.dram_tensor("xt", [128, 16, TOK], bf16, kind="ExternalInput")
    wqkv_d = nc.dram_tensor("wqkv", [128, 16, 768], bf16, kind="ExternalInput")
    wproj_d = nc.dram_tensor("wproj", [128, HC, C], bf16, kind="ExternalInput")
    ident_d = nc.dram_tensor("ident", [128, 128], bf16, kind="ExternalInput")
    ones_d = nc.dram_tensor("ones", [128, 128], bf16, kind="ExternalInput")
    yp_d = nc.dram_tensor("yp", [128, 2, 16, 256], bf16, kind="ExternalOutput")
    kvn_d = nc.dram_tensor("kvn", [128, 2, HC, TOK], bf16, kind="ExternalOutput")

    with tile.TileContext(nc) as tc:
        with (
            tc.tile_pool(name="const", bufs=1) as cpool,
            tc.tile_pool(name="work", bufs=2) as wpool,
            tc.tile_pool(name="ps", bufs=1, space="PSUM") as psum,
        ):
            xt = cpool.tile([128, 16, TOK], bf16)
            wqkv = cpool.tile([128, 16, 768], bf16)
            # interleave xt/wqkv chunk loads so qkv matmuls start early
            for c4 in range(4):
                sl = slice(c4 * 4, c4 * 4 + 4)
                nc.sync.dma_start(xt[:, sl, :], xt_d[:, sl, :])
                nc.sync.dma_start(wqkv[:, sl, :], wqkv_d[:, sl, :])
            wproj = cpool.tile([128, HC, C], bf16)
            nc.sync.dma_start(wproj[:], wproj_d[:])
            ident = cpool.tile([128, 128], bf16)
            nc.sync.dma_start(ident[:], ident_d[:])
            ones = cpool.tile([128, 128], bf16)
            nc.sync.dma_start(ones[:], ones_d[:])
            # last iteration's K, loaded up-front: its scores/exp/l-sum can
            # then run early, so only AV is gated on the final (V) transfer
            ktlast = cpool.tile([128, S], bf16)
            nc.sync.dma_start(ktlast[:], kt_d[NBH - 1])

            qt = cpool.tile([128, HC, TOK], bf16)     # [d, hh, tok]
            ktn = cpool.tile([128, HC, TOK], bf16)
            vtn = cpool.tile([128, HC, TOK], bf16)
            vn = cpool.tile([128, NBH, 128], bf16)    # v_new (t, d), rows 64+ zero
            yt = cpool.tile([128, HC, TOK], bf16)     # yT [d, hh, tok]

            nc.vector.memset(vn[:], 0.0)

            # ---------------- phase A: fused qkv projection (transposed) ----
            # two passes of 3 output col-tiles; ct-outer so matmuls chase the
            # chunked xt/wqkv DMAs instead of waiting for the full weights.
            # pass 0 completes everything head hh=0 needs.
            dsts = [[(qt, 0), (ktn, 0), (vtn, 0)], [(qt, 1), (ktn, 1), (vtn, 1)]]
            jmap = [[0, 2, 4], [1, 3, 5]]
            for p in range(2):
                pstiles = [psum.tile([128, TOK], f32, tag="sc", bufs=4,
                                     name=f"qkvps_{p}_{jj}")
                           for jj in range(3)]
                for ct in range(16):
                    for jj in range(3):
                        nc.tensor.matmul(
                            pstiles[jj][:], wqkv[:, ct, ts(jmap[p][jj], 128)],
                            xt[:, ct, :],
                            start=(ct == 0), stop=(ct == 15),
                        )
                for jj in range(3):
                    dst, hh = dsts[p][jj]
                    nc.scalar.copy(dst[:, hh, :], pstiles[jj][:])
                # transpose this head's v_new tiles: (d, t) -> (t, d)
                for b_ in range(B):
                    i = b_ * HC + p
                    pst = psum.tile([64, 128], bf16, tag="pj", bufs=2)
                    nc.tensor.transpose(pst[:], vtn[:, p, ts(b_, 64)], ident[:])
                    nc.vector.tensor_copy(vn[:64, i, :], pst[:])

            # ---------------- phases B/C: attention + projection ------------
            def emit_proj(half, dma_eng=None, ptag="pj", pbufs=2,
                          defer_dma=False):
                dma_eng = dma_eng or nc.gpsimd
                tok_lo, tok_w = half * 256, 256
                st = wpool.tile([128, 16, tok_w], bf16, tag="ypst")
                for op in range(8):
                    ps = psum.tile([128, 512], f32, tag=ptag, bufs=pbufs)
                    for sub in range(2):
                        ot = op * 2 + sub
                        for cl in range(HC):
                            nc.tensor.matmul(
                                ps[:, ts(sub, tok_w)],
                                wproj[:, cl, ts(ot, 128)],
                                yt[:, cl, tok_lo:tok_lo + tok_w],
                                start=(cl == 0), stop=(cl == HC - 1),
                            )
                    eng = nc.vector.tensor_copy if op % 2 else nc.scalar.copy
                    eng(st[:, ts(op, 2), :], ps[:])
                    if not defer_dma and op % 2 == 1 and op < 7:
                        q = op // 2
                        dma_eng.dma_start(yp_d[:, half, ts(q, 4)],
                                          st[:, ts(q, 4), :])
                if not defer_dma:
                    dma_eng.dma_start(yp_d[:, half, 12:], st[:, 12:, :])
                return st

            if True:
                for i in range(NBH):
                    b_, hh = divmod(i, HC)
                    if i == NBH - 1:
                        ktt = ktlast
                    else:
                        ktt = wpool.tile([128, S], bf16, tag="kt", bufs=3)
                        nc.sync.dma_start(ktt[:], kt_d[i])
                    vt = wpool.tile([128, NT, 128], bf16, tag="v", bufs=3)
                    if i == NBH - 1:
                        # split the final transfer so the tail's AV matmuls
                        # start on the first half while the second streams
                        nc.sync.dma_start(vt[:, :16], v_d[i, :, :16])
                        nc.sync.dma_start(vt[:, 16:], v_d[i, :, 16:])
                    else:
                        nc.sync.dma_start(vt[:], v_d[i])
                    at = wpool.tile([128, NJ * 64], bf16, tag="at")  # [s%128, (j, t)]
                    qv = qt[:, hh, ts(b_, 64)]

                    # new-token tile (33rd) first: it has no DMA dependency
                    ps = psum.tile([128, 512], f32, tag="sc", bufs=4)
                    nc.tensor.matmul(
                        ps[:64, :64], ktn[:, hh, ts(b_, 64)], qv,
                        start=True, stop=True,
                    )
                    nc.scalar.activation(at[:64, NT * 64:], ps[:64, :64], Exp)
                    nc.vector.memset(at[64:, NT * 64:], 0.0)
                    psl = psum.tile([128, 512], f32, tag="l", bufs=1)
                    nc.tensor.matmul(
                        psl[:, :64], ones[:], at[:, NT * 64:],
                        start=True, stop=False,
                    )
                    # scoresT chunks of 8 s-tiles -> exp -> attnT, each chunk
                    # followed by its l-sum accumulation (grouped by j mod 8)
                    for c in range(4):
                        ps = psum.tile([128, 512], f32, tag="sc", bufs=4)
                        for jj in range(8):
                            j = c * 8 + jj
                            nc.tensor.matmul(
                                ps[:, ts(jj, 64)], ktt[:, ts(j, 128)], qv,
                                start=True, stop=True,
                            )
                        nc.scalar.activation(
                            at[:, c * 512:(c + 1) * 512], ps[:], Exp
                        )
                        nc.tensor.matmul(
                            psl[:], ones[:], at[:, c * 512:(c + 1) * 512],
                            start=False, stop=(c == 3),
                        )
                    psy = psum.tile([128, 64], f32, tag="yt", bufs=1)
                    for j in range(NJ):
                        lhsT = vt[:, j, :] if j < NT else vn[:, i, :]
                        nc.tensor.matmul(
                            psy[:], lhsT, at[:, ts(j, 64)],
                            start=(j == 0), stop=(j == NJ - 1),
                        )
                    lsum = wpool.tile([128, 64], f32, tag="lsum")
                    nc.vector.tensor_reduce(
                        lsum[:], psl[:].rearrange("p (j t) -> p t j", j=8),
                        axX, addop,
                    )
                    rl = wpool.tile([128, 64], f32, tag="rl")
                    nc.vector.reciprocal(rl[:], lsum[:])
                    nc.vector.tensor_tensor(
                        yt[:, hh, ts(b_, 64)], psy[:], rl[:], mult
                    )

                    if i == 7:
                        st_a = emit_proj(0, defer_dma=True)  # tokens of b 0..3

                # deferred output transfers on the SP HWDGE FIFO: emitted
                # after every input DMA, FIFO order alone guarantees they
                # follow the last input -- no completion-semaphore wait, and
                # they fill the tail's DMA-idle window
                nc.sync.dma_start(yp_d[:, 0], st_a[:])
                nc.sync.dma_start(kvn_d[:, 0], ktn[:])
                nc.sync.dma_start(kvn_d[:, 1], vtn[:])
                emit_proj(1, dma_eng=nc.sync, ptag="sc", pbufs=4)  # b 4..7

    nc.compile()
    return nc


def _host_prep(x, past_k, past_v, W_attn, W_proj):
    scale = np.float32(1.0 / np.sqrt(HD))
    x2 = np.ascontiguousarray(x.reshape(TOK, C))
    xt = np.ascontiguousarray(
        x2.T.reshape(16, 128, TOK).transpose(1, 0, 2)
    ).astype(bf16np)
    ident = np.eye(128, dtype=bf16np)
    ones = np.ones((128, 128), dtype=bf16np)

    # cast first (halves the bytes moved by the big transposes)
    pk8 = past_k.astype(bf16np)
    pv8 = past_v.astype(bf16np)

    in_maps = []
    for c in range(NCORES):
        h0, h1 = 2 * c, 2 * c + 1
        kt_c = np.ascontiguousarray(
            pk8[:, h0:h1 + 1].transpose(0, 1, 3, 2).reshape(NBH, 128, S)
        )
        v_c = np.ascontiguousarray(
            pv8[:, h0:h1 + 1].reshape(B, HC, NT, 128, 128)
            .transpose(0, 1, 3, 2, 4).reshape(NBH, 128, NT, 128)
        )
        cols = []
        for blk, sc in ((0, scale), (1, None), (2, None)):
            for h in (h0, h1):
                w = W_attn[:, blk * C + h * 128: blk * C + (h + 1) * 128]
                cols.append(w * sc if sc is not None else w)
        wqkv = np.ascontiguousarray(
            np.concatenate(cols, axis=1).reshape(16, 128, 768).transpose(1, 0, 2)
        ).astype(bf16np)
        wproj = np.ascontiguousarray(
            W_proj[h0 * 128:(h1 + 1) * 128, :].reshape(HC, 128, C).transpose(1, 0, 2)
        ).astype(bf16np)
        in_maps.append({
            "kt": kt_c, "v": v_c, "xt": xt, "wqkv": wqkv, "wproj": wproj,
            "ident": ident, "ones": ones,
        })
    return in_maps


def _host_gather(results, past_k, past_v):
    ysum = np.zeros((128, 2, 16, 256), np.float32)
    for c in range(NCORES):
        ysum += results[c]["yp"].astype(np.float32)
    # [p, half, ot, tq] -> yT[ot*128+p, half*256+tq]
    yT = ysum.transpose(2, 0, 1, 3).reshape(C, TOK)
    y = yT.T.reshape(B, T, C)
    y = np.ascontiguousarray(y)

    k_new = np.empty((B, H, T, HD), np.float32)
    v_new = np.empty((B, H, T, HD), np.float32)
    for c in range(NCORES):
        kvn = results[c]["kvn"].astype(np.float32)   # (128, 2, HC, TOK)
        for hh in range(HC):
            k_new[:, 2 * c + hh] = kvn[:, 0, hh].T.reshape(B, T, HD)
            v_new[:, 2 * c + hh] = kvn[:, 1, hh].T.reshape(B, T, HD)
    k = np.concatenate([past_k, k_new], axis=2)
    v = np.concatenate([past_v, v_new], axis=2)
    return y, k, v


def kernel(x, past_k, past_v, W_attn, W_proj):
    from concourse.bass_utils import run_bass_kernel_spmd

    x = np.asarray(x, np.float32)
    past_k = np.asarray(past_k, np.float32)
    past_v = np.asarray(past_v, np.float32)
    W_attn = np.asarray(W_attn, np.float32)
    W_proj = np.asarray(W_proj, np.float32)

    if "nc" not in _prog_cache:
        _prog_cache["nc"] = _build_program()
    nc = _prog_cache["nc"]

    in_maps = _host_prep(x, past_k, past_v, W_attn, W_proj)
    res = run_bass_kernel_spmd(nc, in_maps, core_ids=list(range(NCORES)), trace=False)
    return _host_gather(res.results, past_k, past_v)
